# revision 1
# baseline (speedup 1.0000x reference)
"""Trainium2 Bass kernel for CausalSelfAttention (GQA + qk-rmsnorm + rope + head gating).

Sharding: 8 cores = 2 (batch) x 4 (kv-head groups). Each core computes the
full attention for one batch element and one kv-head group (4 q heads), plus
its slice of the output projection; partial projection outputs are summed on
the host.

Per-core on-device pipeline (all matmuls bf16 with fp32 PSUM accumulation):
  A) fused QKV+gate projection -> rmsnorm stats + rope (DVE/ACT) ->
     DMA-transpose q,k into head-dim-major layout
  B) flash-style causal attention per head in S^T layout:
     S^T = K @ Q^T, P = exp(S/sqrt(d)) (no max subtraction: |logits| <= 11.3),
     diagonal-block masking, Y = P @ [V | 1] (ones column gives the softmax
     denominator for free), per-token normalize * sigmoid gate,
     DMA-transpose y
  C) output projection partial: out = y @ Wproj_slice^T
"""

import numpy as np
import ml_dtypes
from contextlib import ExitStack

import concourse.bass as bass
import concourse.bacc as bacc
import concourse.mybir as mybir
import concourse.tile as tile
from concourse.bass_utils import run_bass_kernel_spmd

BF16 = mybir.dt.bfloat16
F32 = mybir.dt.float32
NPBF = ml_dtypes.bfloat16

B, T, D = 2, 2048, 2048
H, HKV, HD = 16, 4, 128
HALF = HD // 2
NHEAD = H // HKV          # q heads per core (group)
NT = T // 128             # 16 token tiles
NCHUNK = D // 128         # 16 contraction chunks
NQKV = NHEAD * HD + HD + HD + NHEAD   # 512 q + 128 k + 128 v + 4 gate = 772
ROPE_BASE = 10000.0
EPS = float(np.finfo(np.float32).eps)
SM_SCALE = 1.0 / float(np.sqrt(HD))

_CACHE = {}


def _build_program():
    nc = bacc.Bacc("TRN2", target_bir_lowering=False, debug=False,
                   enable_asserts=False, num_devices=8)

    xT_d = nc.dram_tensor("xT", [D, T], BF16, kind="ExternalInput").ap()
    wqkvg_d = nc.dram_tensor("wqkvg", [D, NQKV], BF16, kind="ExternalInput").ap()
    wproj_d = nc.dram_tensor("wproj", [NHEAD * HD, D], BF16, kind="ExternalInput").ap()
    cos_d = nc.dram_tensor("cosd", [T, HALF], F32, kind="ExternalInput").ap()
    sin_d = nc.dram_tensor("sind", [T, HALF], F32, kind="ExternalInput").ap()
    qgain_d = nc.dram_tensor("qgain", [1, NHEAD], F32, kind="ExternalInput").ap()
    gateb_d = nc.dram_tensor("gateb", [1, NHEAD], F32, kind="ExternalInput").ap()
    masks_d = nc.dram_tensor("masks", [128, 4, 512], BF16, kind="ExternalInput").ap()
    out_d = nc.dram_tensor("out", [T, D], F32, kind="ExternalOutput").ap()

    AF = mybir.ActivationFunctionType

    with tile.TileContext(nc) as tc, ExitStack() as ctx:
        consts = ctx.enter_context(tc.tile_pool(name="consts", bufs=1))

        # ---- resident tensors ----
        xT_sb = consts.tile([128, NCHUNK, T], BF16)
        for c in range(NCHUNK):
            nc.sync.dma_start(out=xT_sb[:, c, :], in_=xT_d[c * 128:(c + 1) * 128, :])
        wqkvg_sb = consts.tile([128, NCHUNK, NQKV], BF16)
        for c in range(NCHUNK):
            nc.sync.dma_start(out=wqkvg_sb[:, c, :],
                              in_=wqkvg_d[c * 128:(c + 1) * 128, :])
        wproj_sb = consts.tile([128, NHEAD, D], BF16)
        for h in range(NHEAD):
            nc.sync.dma_start(out=wproj_sb[:, h, :],
                              in_=wproj_d[h * 128:(h + 1) * 128, :])
        cos_sb = consts.tile([128, NT, HALF], F32)
        nc.sync.dma_start(out=cos_sb,
                          in_=cos_d.rearrange("(tt p) i -> p tt i", p=128))
        sin_sb = consts.tile([128, NT, HALF], F32)
        nc.sync.dma_start(out=sin_sb,
                          in_=sin_d.rearrange("(tt p) i -> p tt i", p=128))
        qgain_sb = consts.tile([128, NHEAD], F32)
        nc.sync.dma_start(out=qgain_sb, in_=bass.AP(
            tensor=qgain_d.tensor, offset=qgain_d.offset,
            ap=[[0, 128], [1, NHEAD]]))
        gateb_sb = consts.tile([128, NHEAD], F32)
        nc.sync.dma_start(out=gateb_sb, in_=bass.AP(
            tensor=gateb_d.tensor, offset=gateb_d.offset,
            ap=[[0, 128], [1, NHEAD]]))
        masks_sb = consts.tile([128, 4, 512], BF16)
        nc.sync.dma_start(out=masks_sb, in_=masks_d)

        qT_sb = consts.tile([128, NHEAD, T], BF16)   # head-dim-major q
        kT_sb = consts.tile([128, T], BF16)          # head-dim-major k
        v_sb = consts.tile([128, NT, HD + 1], BF16)  # [t | ones] per ki tile
        nc.vector.memset(v_sb[:, :, HD:HD + 1], 1.0)
        yT_sb = consts.tile([128, NHEAD, T], BF16)   # head-dim-major gated y
        gate_sb = consts.tile([128, NT, NHEAD], F32)
        eps_sb = consts.tile([128, 1], F32)
        nc.vector.memset(eps_sb, EPS)

        # =========== Phase A: QKV + gate, rms stats, rope, transpose ==========
        a_sb = ctx.enter_context(tc.tile_pool(name="phA", bufs=2))
        with tc.tile_pool(name="phA_ps", bufs=2, space="PSUM") as a_ps:
          for tg in range(NT // 4):
            glog_g = a_sb.tile([128, 4, NHEAD], F32, tag="glog_g")
            msq_g = a_sb.tile([128, 4, NHEAD + 1], F32, tag="msq_g")
            qst_g = a_sb.tile([128, 4, NHEAD, HD], BF16, tag="qst_g")
            kst_g = a_sb.tile([128, 4, HD], BF16, tag="kst_g")
            for ti in range(4):
                tt = tg * 4 + ti
                ts = slice(tt * 128, (tt + 1) * 128)
                qkv_a = a_ps.tile([128, 512], F32, tag="qkv_a")
                qkv_b = a_ps.tile([128, NQKV - 512], F32, tag="qkv_b")
                for c in range(NCHUNK):
                    lhs = xT_sb[:, c, ts]
                    nc.tensor.matmul(qkv_a, lhsT=lhs, rhs=wqkvg_sb[:, c, 0:512],
                                     start=(c == 0), stop=(c == NCHUNK - 1))
                    nc.tensor.matmul(qkv_b, lhsT=lhs, rhs=wqkvg_sb[:, c, 512:NQKV],
                                     start=(c == 0), stop=(c == NCHUNK - 1))

                # v tile (+ gate logits)
                nc.vector.tensor_copy(out=v_sb[:, tt, 0:HD], in_=qkv_b[:, 128:256])
                nc.vector.tensor_add(glog_g[:, ti, :], qkv_b[:, 256:260], gateb_sb)

                # rope on q (all 4 heads at once via broadcast cos/sin)
                qa3 = qkv_a.rearrange("p (h d) -> p h d", h=NHEAD)
                x1 = qa3[:, :, 0:HALF]
                x2 = qa3[:, :, HALF:HD]
                cos_t = cos_sb[:, tt, :]
                sin_t = sin_sb[:, tt, :]
                cos_b = bass.AP(tensor=cos_t.tensor, offset=cos_t.offset,
                                ap=[cos_t.ap[0], [0, NHEAD], cos_t.ap[1]])
                sin_b = bass.AP(tensor=sin_t.tensor, offset=sin_t.offset,
                                ap=[sin_t.ap[0], [0, NHEAD], sin_t.ap[1]])
                qrot = qst_g[:, ti, :, :]
                u1 = a_sb.tile([128, NHEAD, HALF], F32, tag="u1")
                u2 = a_sb.tile([128, NHEAD, HALF], F32, tag="u2")
                nc.vector.tensor_mul(u1, x1, cos_b)
                nc.vector.tensor_mul(u2, x2, sin_b)
                nc.vector.tensor_add(qrot[:, :, 0:HALF], u1, u2)
                u3 = a_sb.tile([128, NHEAD, HALF], F32, tag="u3")
                u4 = a_sb.tile([128, NHEAD, HALF], F32, tag="u4")
                nc.vector.tensor_mul(u3, x2, cos_b)
                nc.vector.tensor_mul(u4, x1, sin_b)
                nc.vector.tensor_sub(qrot[:, :, HALF:HD], u3, u4)
                # rms scale + gain, cast to bf16
                # rope on k
                k1 = qkv_b[:, 0:HALF]
                k2 = qkv_b[:, HALF:HD]
                krot = kst_g[:, ti, :]
                w1 = a_sb.tile([128, HALF], F32, tag="w1")
                w2 = a_sb.tile([128, HALF], F32, tag="w2")
                nc.vector.tensor_mul(w1, k1, cos_t)
                nc.vector.tensor_mul(w2, k2, sin_t)
                nc.vector.tensor_add(krot[:, 0:HALF], w1, w2)
                nc.vector.tensor_mul(w1, k2, cos_t)
                nc.vector.tensor_mul(w2, k1, sin_t)
                nc.vector.tensor_sub(krot[:, HALF:HD], w1, w2)

                # mean-square per head from the (norm-preserving) rotated values
                sqscr = a_sb.tile([128, NHEAD, HD], F32, tag="sqscr")
                sqscr_k = a_sb.tile([128, HD], F32, tag="sqscr_k")
                nc.vector.tensor_mul(sqscr, qrot, qrot)
                nc.vector.tensor_reduce(msq_g[:, ti, 0:NHEAD], sqscr,
                                        axis=mybir.AxisListType.X,
                                        op=mybir.AluOpType.add)
                nc.vector.tensor_mul(sqscr_k, krot, krot)
                nc.vector.tensor_reduce(msq_g[:, ti, NHEAD:NHEAD + 1], sqscr_k,
                                        axis=mybir.AxisListType.X,
                                        op=mybir.AluOpType.add)

            # batched scalar math for the 4-tile group (one table load each)
            gslice = gate_sb[:, tg * 4:(tg + 1) * 4, :]
            nc.scalar.activation(
                out=gslice.rearrange("p a b -> p (a b)"),
                in_=glog_g.rearrange("p a b -> p (a b)"), func=AF.Sigmoid)
            rtmp_g = a_sb.tile([128, 4, NHEAD + 1], F32, tag="rtmp_g")
            nc.scalar.activation(out=rtmp_g, in_=msq_g, func=AF.Sqrt,
                                 scale=1.0 / HD, bias=eps_sb)
            rinv_g = a_sb.tile([128, 4, NHEAD + 1], F32, tag="rinv_g")
            nc.vector.reciprocal(rinv_g, rtmp_g)
            rq_g = a_sb.tile([128, 4, NHEAD], F32, tag="rq_g")
            for ti in range(4):
                nc.vector.tensor_mul(rq_g[:, ti, :], rinv_g[:, ti, 0:NHEAD],
                                     qgain_sb)

            for ti in range(4):
                tt = tg * 4 + ti
                ts = slice(tt * 128, (tt + 1) * 128)
                k_stage = a_sb.tile([128, HD], BF16, tag="k_stage")
                nc.vector.tensor_scalar_mul(k_stage, kst_g[:, ti, :],
                                            rinv_g[:, ti, NHEAD:NHEAD + 1])
                q_stage = a_sb.tile([128, NHEAD, HD], BF16, tag="q_stage")
                for h in range(NHEAD):
                    nc.vector.tensor_scalar_mul(q_stage[:, h, :],
                                                qst_g[:, ti, h, :],
                                                rq_g[:, ti, h:h + 1])
                # one combined 4-head transpose (strided 3D out)
                nc.sync.dma_start_transpose(out=qT_sb[:, :, ts], in_=q_stage)
                nc.sync.dma_start_transpose(out=kT_sb[:, ts], in_=k_stage)

        # =========== Phase B + C: attention, projection =======================
        b_sb = ctx.enter_context(tc.tile_pool(name="phB", bufs=3))
        c_sb = ctx.enter_context(tc.tile_pool(name="phC", bufs=3))
        with tc.tile_pool(name="phBC_ps", bufs=2, space="PSUM") as b_ps:
            for qc in range(4):
                qs_slice = slice(qc * 512, (qc + 1) * 512)
                nki = 4 * qc + 4
                for h in range(NHEAD):
                    y01 = b_ps.tile([128, 2, HD + 1], F32, tag="y01")
                    y23 = b_ps.tile([128, 2, HD + 1], F32, tag="y23")
                    for ki in range(nki):
                        m = ki - 4 * qc
                        nq = 512 - 128 * max(m, 0)
                        q_lo = qc * 512 + 128 * max(m, 0)
                        s_ps = b_ps.tile([128, 512], F32, tag="s")
                        nc.tensor.matmul(s_ps[:, 0:nq],
                                         lhsT=kT_sb[:, ki * 128:(ki + 1) * 128],
                                         rhs=qT_sb[:, h, q_lo:(qc + 1) * 512],
                                         start=True, stop=True)
                        p_sb = b_sb.tile([128, 512], BF16, tag="p")
                        nc.scalar.activation(out=p_sb[:, 0:nq], in_=s_ps[:, 0:nq],
                                             func=AF.Exp, scale=SM_SCALE)
                        if m >= 0:
                            nc.vector.tensor_mul(p_sb[:, 0:128], p_sb[:, 0:128],
                                                 masks_sb[:, 0, 0:128])
                        for qs in range(max(m, 0), 4):
                            ytile = y01 if qs < 2 else y23
                            pcol = (qs - max(m, 0)) * 128
                            nc.tensor.matmul(
                                ytile[:, qs % 2, :],
                                lhsT=p_sb[:, pcol:pcol + 128],
                                rhs=v_sb[:, ki, :],
                                start=(ki == 0 and qs % 2 == 0),
                                stop=(ki == 4 * qc + qs and qs % 2 == 1))
                    # normalize + gate + transpose (on the Scalar DMA queue,
                    # off the busy Sync queue)
                    y_stage = b_sb.tile([128, 4, HD], BF16, tag="y_stage")
                    for qs in range(4):
                        ytile = y01 if qs < 2 else y23
                        tt = qc * 4 + qs
                        rd = b_sb.tile([128, 1], F32, tag="rd")
                        nc.vector.reciprocal(rd, ytile[:, qs % 2, HD:HD + 1])
                        sc = b_sb.tile([128, 1], F32, tag="sc")
                        nc.vector.tensor_mul(sc, rd, gate_sb[:, tt, h:h + 1])
                        nc.vector.tensor_scalar_mul(y_stage[:, qs, :],
                                                    ytile[:, qs % 2, 0:HD], sc)
                    yreg = yT_sb[:, h, qc * 512:(qc + 1) * 512]
                    y3d = bass.AP(tensor=yreg.tensor, offset=yreg.offset,
                                  ap=[yreg.ap[0], [128, 4], [1, 128]])
                    nc.scalar.dma_start_transpose(out=y3d, in_=y_stage)

                # Phase C for the token tiles finished by this qc
                for qs in range(4):
                    tt = qc * 4 + qs
                    ts = slice(tt * 128, (tt + 1) * 128)
                    for nch in range(4):
                        o_ps = b_ps.tile([128, 512], F32, tag="o")
                        for h in range(NHEAD):
                            nc.tensor.matmul(o_ps, lhsT=yT_sb[:, h, ts],
                                             rhs=wproj_sb[:, h,
                                                          nch * 512:(nch + 1) * 512],
                                             start=(h == 0), stop=(h == NHEAD - 1))
                        o_st = c_sb.tile([128, 512], F32, tag="o_st")
                        if nch % 2 == 0:
                            nc.scalar.copy(out=o_st, in_=o_ps)
                        else:
                            nc.vector.tensor_copy(out=o_st, in_=o_ps)
                        nc.sync.dma_start(out=out_d[ts, nch * 512:(nch + 1) * 512],
                                          in_=o_st)

    nc.compile()
    return nc


def _get_program():
    if "nc" not in _CACHE:
        _CACHE["nc"] = _build_program()
    return _CACHE["nc"]


def _host_prep(x, Wq, Wk, Wv, Wproj, q_gain, gate_w, gate_b):
    """Build the 8 per-core input maps."""
    f = np.float32
    x = np.asarray(x, f)
    WqT = np.asarray(Wq, f).T.astype(NPBF)       # [D, 2048]
    WkT = np.asarray(Wk, f).T.astype(NPBF)       # [D, 512]
    WvT = np.asarray(Wv, f).T.astype(NPBF)
    WpT = np.ascontiguousarray(np.asarray(Wproj, f).T.astype(NPBF))  # [D, D]
    gwT = np.asarray(gate_w, f).T.astype(NPBF)   # [D, 16]
    q_gain = np.asarray(q_gain, f)
    gate_b = np.asarray(gate_b, f)

    inv_freq = 1.0 / (ROPE_BASE ** (np.arange(0, HD, 2, dtype=f) / HD))
    tpos = np.arange(T, dtype=f)
    freqs = np.outer(tpos, inv_freq)
    cos = np.cos(freqs).astype(f)
    sin = np.sin(freqs).astype(f)

    kloc = np.arange(128)[:, None]
    qloc = np.arange(512)[None, :]
    masks = np.stack([(qloc >= kloc + 128 * m) for m in range(4)], axis=1)
    masks = masks.astype(NPBF)                   # [128, 4, 512]

    xT = [np.ascontiguousarray(x[b].T).astype(NPBF) for b in range(B)]

    in_maps = []
    for core in range(8):
        b, g = divmod(core, 4)
        wqkvg = np.concatenate([
            WqT[:, 512 * g:512 * (g + 1)],
            WkT[:, 128 * g:128 * (g + 1)],
            WvT[:, 128 * g:128 * (g + 1)],
            gwT[:, NHEAD * g:NHEAD * (g + 1)],
        ], axis=1)                               # [D, 772]
        in_maps.append({
            "xT": xT[b],
            "wqkvg": np.ascontiguousarray(wqkvg),
            "wproj": np.ascontiguousarray(WpT[512 * g:512 * (g + 1), :]),
            "cosd": cos,
            "sind": sin,
            "qgain": np.ascontiguousarray(q_gain[NHEAD * g:NHEAD * (g + 1)][None, :]),
            "gateb": np.ascontiguousarray(gate_b[NHEAD * g:NHEAD * (g + 1)][None, :]),
            "masks": masks,
        })
    return in_maps


def kernel(**inputs):
    nc = _get_program()
    in_maps = _host_prep(**inputs)
    res = run_bass_kernel_spmd(nc, in_maps, list(range(8)))
    parts = [r["out"] for r in res.results]
    out = np.empty((B, T, D), np.float32)
    for b in range(B):
        out[b] = parts[4 * b] + parts[4 * b + 1] + parts[4 * b + 2] + parts[4 * b + 3]
    return out



# revision 10
# speedup vs baseline: 1.0963x; 1.0963x over previous
"""Trainium2 Bass kernel for CausalSelfAttention (GQA + qk-rmsnorm + rope + head gating).

Sharding: 8 cores = 2 (batch) x 4 (kv-head groups). Each core computes the
full attention for one batch element and one kv-head group (4 q heads), plus
its slice of the output projection; partial projection outputs are summed on
the host (bf16 partials, fp32 accumulation).

Per-core pipeline, merged across phases per 4-token-tile group g:
  A) fused QKV+gate projection for tiles 4g..4g+3 -> bf16 cast ->
     full-width rope (3 DVE ops/group, swap-AP + pre-signed sin table) ->
     rms stats (mul+reduce) -> scale -> DMA-transpose q,k to head-dim-major
  B) causal attention for q chunk g in S^T layout, software-pipelined:
     S^T = K @ Q^T (PE), P = exp(S/sqrt(d)) (ACT), diagonal mask (DVE),
     Y = P @ [V | 1] (PE, ones column = softmax denominator), with the
     previous chunk's output-projection matmuls interleaved into the
     tensor queue to fill exp-latency stalls
  C) output projection partial in bf16, 256KB stores
k-side elementwise work (rope/stats/scale/v-copy) runs on GpSimd to keep
the DVE under the tensor-engine roofline.
"""

import numpy as np
import ml_dtypes
from contextlib import ExitStack

import concourse.bass as bass
import concourse.bacc as bacc
import concourse.mybir as mybir
import concourse.tile as tile
from concourse.bass_utils import run_bass_kernel_spmd

BF16 = mybir.dt.bfloat16
F32 = mybir.dt.float32
NPBF = ml_dtypes.bfloat16

B, T, D = 2, 2048, 2048
H, HKV, HD = 16, 4, 128
HALF = HD // 2
NHEAD = H // HKV          # q heads per core (group)
NT = T // 128             # 16 token tiles
NG = 4                    # 4-tile groups
NCHUNK = D // 128         # 16 contraction chunks
NKVG = HD + HD + NHEAD    # 128 k + 128 v + 4 gate = 260
ROPE_BASE = 10000.0
EPS = float(np.finfo(np.float32).eps)
SM_SCALE = 1.0 / float(np.sqrt(HD))

_CACHE = {}


def _build_program():
    nc = bacc.Bacc("TRN2", target_bir_lowering=False, debug=False,
                   enable_asserts=False, num_devices=8)

    xT_d = nc.dram_tensor("xT", [D, T], BF16, kind="ExternalInput").ap()
    wq_d = nc.dram_tensor("wq", [D, NHEAD * HD], BF16, kind="ExternalInput").ap()
    wkvg_d = nc.dram_tensor("wkvg", [D, NKVG], BF16, kind="ExternalInput").ap()
    wproj_d = nc.dram_tensor("wproj", [NHEAD * HD, D], BF16, kind="ExternalInput").ap()
    cos_d = nc.dram_tensor("cosd", [T, HD], BF16, kind="ExternalInput").ap()
    sin_d = nc.dram_tensor("sind", [T, HD], BF16, kind="ExternalInput").ap()
    qgain_d = nc.dram_tensor("qgain", [1, NHEAD], F32, kind="ExternalInput").ap()
    gateb_d = nc.dram_tensor("gateb", [1, NHEAD], F32, kind="ExternalInput").ap()
    mask_d = nc.dram_tensor("masks", [128, 128], BF16, kind="ExternalInput").ap()
    out_d = nc.dram_tensor("out", [T, D], BF16, kind="ExternalOutput").ap()

    AF = mybir.ActivationFunctionType

    with tile.TileContext(nc) as tc, ExitStack() as ctx:
        consts = ctx.enter_context(tc.tile_pool(name="consts", bufs=1))

        # ---- resident tensors (loads interleaved so compute starts early) ----
        xT_sb = consts.tile([128, NCHUNK, T], BF16)
        wq_sb = consts.tile([128, NCHUNK, NHEAD * HD], BF16)
        wkvg_sb = consts.tile([128, NCHUNK, NKVG], BF16)
        for c in range(NCHUNK):
            cs = slice(c * 128, (c + 1) * 128)
            nc.sync.dma_start(out=xT_sb[:, c, :], in_=xT_d[cs, :])
            nc.sync.dma_start(out=wq_sb[:, c, :], in_=wq_d[cs, :])
            nc.sync.dma_start(out=wkvg_sb[:, c, :], in_=wkvg_d[cs, :])
        wproj_sb = consts.tile([128, NHEAD, D], BF16)
        for h in range(NHEAD):
            nc.scalar.dma_start(out=wproj_sb[:, h, :],
                                in_=wproj_d[h * 128:(h + 1) * 128, :])
        cos_sb = consts.tile([128, NT, HD], BF16)
        nc.scalar.dma_start(out=cos_sb,
                            in_=cos_d.rearrange("(tt p) i -> p tt i", p=128))
        sin_sb = consts.tile([128, NT, HD], BF16)
        nc.scalar.dma_start(out=sin_sb,
                            in_=sin_d.rearrange("(tt p) i -> p tt i", p=128))
        qgain_sb = consts.tile([128, NHEAD], F32)
        nc.scalar.dma_start(out=qgain_sb, in_=bass.AP(
            tensor=qgain_d.tensor, offset=qgain_d.offset,
            ap=[[0, 128], [1, NHEAD]]))
        gateb_sb = consts.tile([128, NHEAD], F32)
        nc.scalar.dma_start(out=gateb_sb, in_=bass.AP(
            tensor=gateb_d.tensor, offset=gateb_d.offset,
            ap=[[0, 128], [1, NHEAD]]))
        mask_sb = consts.tile([128, 128], BF16)
        nc.scalar.dma_start(out=mask_sb, in_=mask_d)

        qT_sb = consts.tile([128, NHEAD, T], BF16)   # head-dim-major q
        kT_sb = consts.tile([128, T], BF16)          # head-dim-major k
        v_sb = consts.tile([128, NT, HD + 1], BF16)  # [v | ones] per ki tile
        nc.vector.memset(v_sb[:, :, HD:HD + 1], 1.0)
        yT_sb = consts.tile([128, NHEAD, T], BF16)   # head-dim-major gated y
        gate_sb = consts.tile([128, NT, NHEAD], F32)
        eps_sb = consts.tile([128, 1], F32)
        nc.vector.memset(eps_sb, EPS)

        a_sb = ctx.enter_context(tc.tile_pool(name="phA", bufs=2))
        b_sb = ctx.enter_context(tc.tile_pool(name="phB", bufs=3))
        ps = ctx.enter_context(tc.tile_pool(name="ps", bufs=1, space="PSUM"))

        def swap_halves(ap3):
            """[p, n, HD] AP -> same with the two HD/2 halves swapped."""
            return bass.AP(tensor=ap3.tensor, offset=ap3.offset + HALF,
                           ap=[ap3.ap[0], ap3.ap[1], [-HALF, 2], [1, HALF]])

        def split_halves(ap3):
            """[p, n, HD] AP -> [p, n, 2, HD/2] (no swap), to match shapes."""
            return bass.AP(tensor=ap3.tensor, offset=ap3.offset,
                           ap=[ap3.ap[0], ap3.ap[1], [HALF, 2], [1, HALF]])

        # ---------------- proj thunk generator (phase C) ----------------
        def make_proj(qc):
            def gen():
                for half in range(2):          # nch pairs (0,1) and (2,3)
                    for qs in range(4):
                        tt = qc * 4 + qs
                        ts = slice(tt * 128, (tt + 1) * 128)
                        o_st = b_sb.tile([128, 1024], BF16, tag="o_st")
                        for sub in range(2):
                            nch = half * 2 + sub
                            o_ps = ps.tile([128, 512], F32, tag="o",
                                           bufs=2)
                            for h in range(NHEAD):
                                yield lambda o_ps=o_ps, h=h, ts=ts, nch=nch: \
                                    nc.tensor.matmul(
                                        o_ps, lhsT=yT_sb[:, h, ts],
                                        rhs=wproj_sb[:, h,
                                                     nch * 512:(nch + 1) * 512],
                                        start=(h == 0), stop=(h == NHEAD - 1))
                            yield lambda o_ps=o_ps, o_st=o_st, sub=sub: \
                                nc.vector.tensor_copy(
                                    out=o_st[:, sub * 512:(sub + 1) * 512],
                                    in_=o_ps)
                        yield lambda o_st=o_st, ts=ts, half=half: \
                            nc.sync.dma_start(
                                out=out_d[ts, half * 1024:(half + 1) * 1024],
                                in_=o_st)
            return gen()

        def drain(gen, n):
            if gen is None:
                return
            for _ in range(n):
                try:
                    next(gen)()
                except StopIteration:
                    return

        proj_gen = None

        for g in range(NG):
            # ================= Phase A: tiles 4g .. 4g+3 =================
            qa_g = a_sb.tile([128, 4, NHEAD * HD], BF16, tag="qa_g")
            kb_g = a_sb.tile([128, 4, NKVG], BF16, tag="kb_g")
            qst_g = a_sb.tile([128, 4, NHEAD, HD], BF16, tag="qst_g")
            kst_g = a_sb.tile([128, 4, HD], BF16, tag="kst_g")
            uk_g = a_sb.tile([128, 4, HD], BF16, tag="uk_g")
            sq_g = a_sb.tile([128, 4, NHEAD * HD], BF16, tag="sq_g")
            sqk_g = a_sb.tile([128, 4, HD], BF16, tag="sqk_g")
            glog_g = a_sb.tile([128, 4, NHEAD], F32, tag="glog_g")
            msq_g = a_sb.tile([128, 4, NHEAD + 1], F32, tag="msq_g")
            rtmp_g = a_sb.tile([128, 4, NHEAD + 1], F32, tag="rtmp_g")
            rinv_g = a_sb.tile([128, 4, NHEAD + 1], F32, tag="rinv_g")
            rq_g = a_sb.tile([128, 4, NHEAD], F32, tag="rq_g")

            for ti in range(4):
                tt = g * 4 + ti
                ts = slice(tt * 128, (tt + 1) * 128)
                q_ps = ps.tile([128, 512], F32, tag="qkv", bufs=2)
                for c in range(NCHUNK):
                    nc.tensor.matmul(q_ps, lhsT=xT_sb[:, c, ts],
                                     rhs=wq_sb[:, c, :],
                                     start=(c == 0), stop=(c == NCHUNK - 1))
                nc.vector.tensor_copy(out=qa_g[:, ti, :], in_=q_ps)
                b_ps = ps.tile([128, 512], F32, tag="qkv", bufs=2)
                for c in range(NCHUNK):
                    nc.tensor.matmul(b_ps[:, 0:NKVG], lhsT=xT_sb[:, c, ts],
                                     rhs=wkvg_sb[:, c, :],
                                     start=(c == 0), stop=(c == NCHUNK - 1))
                nc.vector.tensor_copy(out=kb_g[:, ti, :], in_=b_ps[:, 0:NKVG])
                nc.vector.tensor_copy(out=v_sb[:, tt, 0:HD],
                                      in_=kb_g[:, ti, HD:2 * HD])

            # gate logits (+bias broadcast over the 4 tiles)
            gateb_b = bass.AP(tensor=gateb_sb.tensor, offset=gateb_sb.offset,
                              ap=[gateb_sb.ap[0], [0, 4], [1, NHEAD]])
            nc.gpsimd.tensor_add(glog_g, kb_g[:, :, 2 * HD:2 * HD + NHEAD],
                                 gateb_b)

            # ---- q rope per tile: full-width pre-signed tables, h-bcast ----
            for ti in range(4):
                tt = g * 4 + ti
                u_t = a_sb.tile([128, NHEAD, HD], BF16, tag="u_t", bufs=2)
                qa_t = qa_g[:, ti, :]
                qa_h = bass.AP(tensor=qa_t.tensor, offset=qa_t.offset,
                               ap=[qa_t.ap[0], [HD, NHEAD], [1, HD]])
                qst_t = qst_g[:, ti, :, :]
                cos_t = cos_sb[:, tt, :]
                cos_h = bass.AP(tensor=cos_t.tensor, offset=cos_t.offset,
                                ap=[cos_t.ap[0], [0, NHEAD], [1, HD]])
                sin_t = sin_sb[:, tt, :]
                sin_h = bass.AP(tensor=sin_t.tensor, offset=sin_t.offset,
                                ap=[sin_t.ap[0], [0, NHEAD], [1, HD]])
                nc.vector.tensor_mul(qst_t, qa_h, cos_h)
                nc.vector.tensor_mul(split_halves(u_t), swap_halves(qa_h),
                                     split_halves(sin_h))
                nc.vector.tensor_add(qst_t, qst_t, u_t)

            # ---- k rope on gpsimd ----
            cos_t = cos_sb[:, g * 4:(g + 1) * 4, :]
            sin_t = sin_sb[:, g * 4:(g + 1) * 4, :]
            kin = kb_g[:, :, 0:HD]
            nc.gpsimd.tensor_mul(kst_g, kin, cos_t)
            nc.gpsimd.tensor_mul(split_halves(uk_g), swap_halves(kin),
                                 split_halves(sin_t))
            nc.gpsimd.tensor_add(kst_g, kst_g, uk_g)

            # ---- mean-square (rope preserves norms; use rotated values) ----
            nc.vector.tensor_mul(sq_g, qst_g, qst_g)
            msq_q = bass.AP(tensor=msq_g.tensor, offset=msq_g.offset,
                            ap=[msq_g.ap[0], [NHEAD + 1, 4], [1, NHEAD]])
            sq_red = bass.AP(tensor=sq_g.tensor, offset=sq_g.offset,
                             ap=[sq_g.ap[0], [HD, 16], [1, HD]])
            nc.vector.tensor_reduce(msq_q, sq_red,
                                    axis=mybir.AxisListType.X,
                                    op=mybir.AluOpType.add)
            nc.gpsimd.tensor_mul(sqk_g, kst_g, kst_g)
            nc.vector.tensor_reduce(msq_g[:, :, NHEAD:NHEAD + 1], sqk_g,
                                    axis=mybir.AxisListType.X,
                                    op=mybir.AluOpType.add)

            # ---- batched scalar math ----
            nc.scalar.activation(
                out=gate_sb[:, g * 4:(g + 1) * 4, :].rearrange(
                    "p a b -> p (a b)"),
                in_=glog_g.rearrange("p a b -> p (a b)"), func=AF.Sigmoid)
            nc.scalar.activation(out=rtmp_g, in_=msq_g, func=AF.Sqrt,
                                 scale=1.0 / HD, bias=eps_sb)
            nc.vector.reciprocal(rinv_g, rtmp_g)
            qgain_b = bass.AP(tensor=qgain_sb.tensor, offset=qgain_sb.offset,
                              ap=[qgain_sb.ap[0], [0, 4], [1, NHEAD]])
            nc.vector.tensor_mul(rq_g, rinv_g[:, :, 0:NHEAD], qgain_b)

            # ---- scale into fresh staging tiles + transpose ----
            kfin_g = a_sb.tile([128, 4, HD], BF16, tag="kfin_g")
            rk_b = bass.AP(tensor=rinv_g.tensor,
                           offset=rinv_g.offset + NHEAD,
                           ap=[rinv_g.ap[0], [NHEAD + 1, 4], [0, HD]])
            nc.vector.tensor_mul(kfin_g, kst_g, rk_b)
            for ti in range(4):
                tt = g * 4 + ti
                ts = slice(tt * 128, (tt + 1) * 128)
                q_fin = a_sb.tile([128, NHEAD, HD], BF16, tag="q_fin",
                                  bufs=2)
                rq_t = rq_g[:, ti, :]
                rq_b = bass.AP(tensor=rq_t.tensor, offset=rq_t.offset,
                               ap=[rq_t.ap[0], [1, NHEAD], [0, HD]])
                nc.vector.tensor_mul(q_fin, qst_g[:, ti, :, :], rq_b)
                yreg = qT_sb[:, :, ts]
                q3d = bass.AP(tensor=yreg.tensor, offset=yreg.offset,
                              ap=[yreg.ap[0], [T, NHEAD], [1, 128]])
                nc.sync.dma_start_transpose(out=q3d, in_=q_fin)
            kreg = kT_sb[:, g * 512:(g + 1) * 512]
            k3d = bass.AP(tensor=kreg.tensor, offset=kreg.offset,
                          ap=[kreg.ap[0], [128, 4], [1, 128]])
            nc.sync.dma_start_transpose(out=k3d, in_=kfin_g)

            # ============== Phase B: attention for q chunk g ==============
            qc = g
            nki = 4 * qc + 4
            n_iters = NHEAD * nki
            per_iter = 0 if proj_gen is None else (96 + n_iters - 1) // n_iters
            for h in range(NHEAD):
                y01 = ps.tile([128, 2, HD + 1], F32, tag="y01", bufs=1)
                y23 = ps.tile([128, 2, HD + 1], F32, tag="y23", bufs=1)
                prev = None
                for ki in range(nki):
                    m = ki - 4 * qc
                    nq = 512 - 128 * max(m, 0)
                    q_lo = qc * 512 + 128 * max(m, 0)
                    s_ps = ps.tile([128, 512], F32, tag="s", bufs=2)
                    nc.tensor.matmul(s_ps[:, 0:nq],
                                     lhsT=kT_sb[:, ki * 128:(ki + 1) * 128],
                                     rhs=qT_sb[:, h, q_lo:(qc + 1) * 512],
                                     start=True, stop=True)
                    p_sb = b_sb.tile([128, 512], BF16, tag="p")
                    nc.scalar.activation(out=p_sb[:, 0:nq], in_=s_ps[:, 0:nq],
                                         func=AF.Exp, scale=SM_SCALE)
                    if m >= 0:
                        nc.gpsimd.tensor_mul(p_sb[:, 0:128], p_sb[:, 0:128],
                                             mask_sb)
                    if prev is not None:
                        _issue_pv(nc, prev, y01, y23, v_sb, qc)
                    drain(proj_gen, per_iter)
                    prev = (ki, p_sb)
                _issue_pv(nc, prev, y01, y23, v_sb, qc)

                # normalize + gate -> bf16 staging, transpose on scalar queue
                rd4 = b_sb.tile([128, 4], F32, tag="rd4")
                nc.vector.reciprocal(rd4[:, 0:2], bass.AP(
                    tensor=y01.tensor, offset=y01.offset + HD,
                    ap=[y01.ap[0], [HD + 1, 2]]))
                nc.vector.reciprocal(rd4[:, 2:4], bass.AP(
                    tensor=y23.tensor, offset=y23.offset + HD,
                    ap=[y23.ap[0], [HD + 1, 2]]))
                sc4 = b_sb.tile([128, 4], F32, tag="sc4")
                gslice = bass.AP(
                    tensor=gate_sb.tensor,
                    offset=gate_sb.offset + (4 * qc) * NHEAD + h,
                    ap=[gate_sb.ap[0], [NHEAD, 4], [1, 1]])
                nc.vector.tensor_mul(sc4, rd4, gslice)
                y_stage = b_sb.tile([128, 4, HD], BF16, tag="y_stage")
                for qs in range(4):
                    ytile = y01 if qs < 2 else y23
                    nc.vector.tensor_scalar_mul(y_stage[:, qs, :],
                                                ytile[:, qs % 2, 0:HD],
                                                sc4[:, qs:qs + 1])
                yreg = yT_sb[:, h, qc * 512:(qc + 1) * 512]
                y3d = bass.AP(tensor=yreg.tensor, offset=yreg.offset,
                              ap=[yreg.ap[0], [128, 4], [1, 128]])
                nc.scalar.dma_start_transpose(out=y3d, in_=y_stage)

            drain(proj_gen, 10000)
            proj_gen = make_proj(qc)

        drain(proj_gen, 10000)

    nc.compile()
    return nc


def _issue_pv(nc, prev, y01, y23, v_sb, qc):
    ki, p_sb = prev
    m = ki - 4 * qc
    for qs in range(max(m, 0), 4):
        ytile = y01 if qs < 2 else y23
        pcol = (qs - max(m, 0)) * 128
        nc.tensor.matmul(
            ytile[:, qs % 2, :],
            lhsT=p_sb[:, pcol:pcol + 128],
            rhs=v_sb[:, ki, :],
            start=(ki == 0 and qs % 2 == 0),
            stop=(ki == 4 * qc + qs and qs % 2 == 1))


def _get_program():
    if "nc" not in _CACHE:
        _CACHE["nc"] = _build_program()
    return _CACHE["nc"]


def _host_prep(x, Wq, Wk, Wv, Wproj, q_gain, gate_w, gate_b):
    """Build the 8 per-core input maps."""
    f = np.float32
    x = np.asarray(x, f)
    WqT = np.asarray(Wq, f).T.astype(NPBF)       # [D, 2048]
    WkT = np.asarray(Wk, f).T.astype(NPBF)       # [D, 512]
    WvT = np.asarray(Wv, f).T.astype(NPBF)
    WpT = np.ascontiguousarray(np.asarray(Wproj, f).T.astype(NPBF))  # [D, D]
    gwT = np.asarray(gate_w, f).T.astype(NPBF)   # [D, 16]
    q_gain = np.asarray(q_gain, f)
    gate_b = np.asarray(gate_b, f)

    inv_freq = 1.0 / (ROPE_BASE ** (np.arange(0, HD, 2, dtype=f) / HD))
    tpos = np.arange(T, dtype=f)
    freqs = np.outer(tpos, inv_freq)             # [T, 64]
    cosF = np.concatenate([np.cos(freqs), np.cos(freqs)], axis=1)
    sinF = np.concatenate([np.sin(freqs), -np.sin(freqs)], axis=1)
    cosF = cosF.astype(NPBF)                     # [T, 128]
    sinF = sinF.astype(NPBF)

    kloc = np.arange(128)[:, None]
    qloc = np.arange(128)[None, :]
    mask = (qloc >= kloc).astype(NPBF)           # [128, 128]

    xT = [np.ascontiguousarray(x[b].T).astype(NPBF) for b in range(B)]

    in_maps = []
    for core in range(8):
        b, g = divmod(core, 4)
        wkvg = np.concatenate([
            WkT[:, 128 * g:128 * (g + 1)],
            WvT[:, 128 * g:128 * (g + 1)],
            gwT[:, NHEAD * g:NHEAD * (g + 1)],
        ], axis=1)                               # [D, 260]
        in_maps.append({
            "xT": xT[b],
            "wq": np.ascontiguousarray(WqT[:, 512 * g:512 * (g + 1)]),
            "wkvg": np.ascontiguousarray(wkvg),
            "wproj": np.ascontiguousarray(WpT[512 * g:512 * (g + 1), :]),
            "cosd": cosF,
            "sind": sinF,
            "qgain": np.ascontiguousarray(q_gain[NHEAD * g:NHEAD * (g + 1)][None, :]),
            "gateb": np.ascontiguousarray(gate_b[NHEAD * g:NHEAD * (g + 1)][None, :]),
            "masks": mask,
        })
    return in_maps


def kernel(**inputs):
    nc = _get_program()
    in_maps = _host_prep(**inputs)
    res = run_bass_kernel_spmd(nc, in_maps, list(range(8)))
    parts = [r["out"] for r in res.results]
    out = np.empty((B, T, D), np.float32)
    for b in range(B):
        out[b] = (parts[4 * b].astype(np.float32)
                  + parts[4 * b + 1].astype(np.float32)
                  + parts[4 * b + 2].astype(np.float32)
                  + parts[4 * b + 3].astype(np.float32))
    return out
